# revision 7
# baseline (speedup 1.0000x reference)
"""Trainium2 Bass kernel for nn_AtomicNeuralNetwork (species-routed per-atom MLP).

Math (per frame n, atom a with species s = numbers[a]):
    h1 = silu(W1[s].T x + b1[s]);  h2 = silu(W2[s].T h1 + b2[s]);  out = W3[s].T h2 + b3[s]
Shapes: N=4096 frames, A=256 atoms, D_IN=39, H=50, S=8 species.

Strategy (v2 — K-stacked atom pairs):
  - Data parallel over frames: 512 frames per NeuronCore x 8 cores.
  - Atoms are species-sorted and PAIRED (128 pairs). Each moving column
    carries BOTH atoms' descriptors stacked on the contraction axis
    (rows 0:39 atom-a, 39:78 atom-b, row 78 = 1.0 for the bias), and the
    stationary weight is the block-diagonal [W1a 0; 0 W1b; b1a b1b]
    ([79, 100]). One 512-frame column stream therefore computes BOTH
    atoms' 50 channels -> half the PE streaming time of per-atom matmuls.
  - Layer 2 is the same with [101, 100] block-diag (row 100 = biases, fed
    by a constant 1.0 baked into row 100 of the h tiles, which the silu
    ACT never touches since it writes partitions 0:100 only).
  - All biases are folded into the matmuls, so the silu activations are
    bias-free ACT instructions over [100, 3*512] PSUM tiles (FD=1536).
  - Layer 3 uses per-pair one-hot-column stationaries [101, 64] (cols
    2k/2k+1 = w3a|b3a / w3b|b3b of pair slot k) accumulating 32 pairs
    into one shared PSUM bank [64, 512]; 4 such output groups per core
    are DMA'd straight PSUM->DRAM (no vector-engine pass at all).
  - PSUM: 2x [128,1536] ping-pong stage tiles (L2 reuses L1's banks after
    ACT1 consumed them) + 2x [128,512] output banks = exactly 8 banks.
  - Everything bf16 on the matmul path; desc packed/downcast on host.
"""

import sys

for _p in ("/opt/trn_rl_repo",):
    if _p not in sys.path:
        sys.path.append(_p)

import contextlib

import numpy as np
import ml_dtypes

import concourse.bass as bass  # noqa: F401
import concourse.mybir as mybir
import concourse.tile as tile
from concourse import bacc
from concourse import bass_utils

N, A, D, H, S = 4096, 256, 39, 50, 8
NCORES = 8
NF = N // NCORES            # frames per core
NPAIR = A // 2              # 128
PDG = 16                    # pairs per desc DMA group
NDG = NPAIR // PDG          # 8
POG = 16                    # pairs per output group (one PSUM bank, M=32)
NOG = NPAIR // POG          # 8
UNIT = 3                    # pairs per pipeline unit (PSUM stage = 3 banks)
KD = 2 * D + 1              # 79: stacked descriptor rows + ones row
KH = 2 * H + 1              # 101: stacked hidden rows + ones row
MM_DT = mybir.dt.bfloat16
NP_MM = ml_dtypes.bfloat16

LAST = {}


def _make_pairs(species):
    """Species-sort atoms, pair consecutively. Returns (pa, pb, tidx,
    type_list) where pair j = atoms (pa[j], pb[j]) with species type
    type_list[tidx[j]] = (sa, sb)."""
    order = np.argsort(species, kind="stable")
    pa, pb = order[0::2], order[1::2]
    types = [(int(species[a]), int(species[b])) for a, b in zip(pa, pb)]
    type_list = sorted(set(types))
    t_of = {t: i for i, t in enumerate(type_list)}
    tidx = np.array([t_of[t] for t in types])
    return pa, pb, tidx, type_list


def _build_program(tidx, ntype, repeat=0):
    nc = bacc.Bacc("TRN2", target_bir_lowering=False, debug=False)

    desc_in = nc.dram_tensor("desc_in", [NDG, KD, PDG * NF], MM_DT, kind="ExternalInput")
    w1_in = nc.dram_tensor("w1_in", [128, 100 * ntype], MM_DT, kind="ExternalInput")
    w2_in = nc.dram_tensor("w2_in", [128, 100 * ntype], MM_DT, kind="ExternalInput")
    w3_in = nc.dram_tensor("w3_in", [128, 32 * NPAIR], MM_DT, kind="ExternalInput")
    out = nc.dram_tensor("out", [NOG, 32, NF], mybir.dt.float32, kind="ExternalOutput")

    Silu = mybir.ActivationFunctionType.Silu
    F32 = mybir.dt.float32

    units = []
    p = 0
    while p < NPAIR:
        nu = min(UNIT, NPAIR - p)
        units.append((p, nu))
        p += nu

    with tile.TileContext(nc) as tc:
        with (
            tc.tile_pool(name="const", bufs=1) as cpool,
            tc.tile_pool(name="dt", bufs=2) as dpool,
            tc.tile_pool(name="h1p", bufs=3) as h1pool,
            tc.tile_pool(name="h2p", bufs=3) as h2pool,
            tc.tile_pool(name="ps", bufs=2, space="PSUM") as pspool,
            tc.tile_pool(name="op", bufs=2, space="PSUM") as opool,
            tc.tile_pool(name="ob", bufs=2) as obpool,
        ):
            w1t = cpool.tile([128, 100 * ntype], MM_DT)
            w2t = cpool.tile([128, 100 * ntype], MM_DT)
            w3t = cpool.tile([128, 32 * NPAIR], MM_DT)
            for t, src in ((w1t, w1_in), (w2t, w2_in), (w3t, w3_in)):
                nc.sync.dma_start(t[:], src[:])

            # bake the constant 1.0 bias-row into every h-pool slot once;
            # the ACTs only ever write partitions 0:100, so it persists.
            ones_rows = []
            for pool in (h1pool, h2pool):
                for _ in range(3):
                    t = pool.tile([128, UNIT * NF], MM_DT, tag="hh")
                    nc.vector.memset(t[96:101, :], 1.0)
                    ones_rows.append(t)

            loop_cm = tc.For_i(0, repeat, 1) if repeat else contextlib.nullcontext()
            with loop_cm:
                dt_tiles = {}
                next_g = 0

                def want_group(g):
                    nonlocal next_g
                    while next_g <= min(g + 1, NDG - 1):
                        t = dpool.tile([128, PDG * NF], MM_DT, tag="dt")
                        nc.gpsimd.dma_start(t[0:KD, :], desc_in[next_g, :, :])
                        dt_tiles[next_g] = t
                        next_g += 1

                o_tile = None
                for (p0, nu) in units:
                    fd = nu * NF
                    want_group(p0 // PDG)

                    ps = pspool.tile([128, UNIT * NF], F32, tag="ps")
                    for k in range(nu):
                        p = p0 + k
                        g, off = divmod(p, PDG)
                        want_group(g)
                        tcol = 100 * int(tidx[p])
                        nc.tensor.matmul(
                            ps[0:100, k * NF:(k + 1) * NF],
                            w1t[0:KD, tcol:tcol + 100],
                            dt_tiles[g][0:KD, off * NF:(off + 1) * NF],
                            start=True, stop=True)

                    h1 = h1pool.tile([128, UNIT * NF], MM_DT, tag="hh")
                    nc.scalar.activation(h1[0:100, 0:fd], ps[0:100, 0:fd], Silu)

                    for k in range(nu):
                        p = p0 + k
                        tcol = 100 * int(tidx[p])
                        nc.tensor.matmul(
                            ps[0:100, k * NF:(k + 1) * NF],
                            w2t[0:KH, tcol:tcol + 100],
                            h1[0:KH, k * NF:(k + 1) * NF],
                            start=True, stop=True)

                    h2 = h2pool.tile([128, UNIT * NF], MM_DT, tag="hh")
                    nc.scalar.activation(h2[0:100, 0:fd], ps[0:100, 0:fd], Silu)

                    for k in range(nu):
                        p = p0 + k
                        og, slot = divmod(p, POG)
                        if slot == 0:
                            o_tile = opool.tile([128, NF], F32, tag="o")
                        nc.tensor.matmul(
                            o_tile[0:32, :],
                            w3t[0:KH, 32 * p:32 * p + 32],
                            h2[0:KH, k * NF:(k + 1) * NF],
                            start=(slot == 0), stop=(slot == POG - 1))
                        if slot == POG - 1:
                            o_sb = obpool.tile([128, NF], F32, tag="ob")
                            nc.vector.tensor_copy(o_sb[0:32, :], o_tile[0:32, :])
                            nc.sync.dma_start(out[og, :, :], o_sb[0:32, :])

    nc.compile()
    return nc


def _host_inputs(desc, numbers, W1, b1, W2, b2, W3, b3):
    desc = np.asarray(desc, dtype=np.float32)
    numbers = np.asarray(numbers).astype(np.int64)
    W1 = np.asarray(W1, np.float32); b1 = np.asarray(b1, np.float32)
    W2 = np.asarray(W2, np.float32); b2 = np.asarray(b2, np.float32)
    W3 = np.asarray(W3, np.float32); b3 = np.asarray(b3, np.float32)

    pa, pb, tidx, type_list = _make_pairs(numbers)
    T = len(type_list)

    w1blk = np.zeros((128, 100 * T), np.float32)
    w2blk = np.zeros((128, 100 * T), np.float32)
    for t, (sa, sb) in enumerate(type_list):
        c = 100 * t
        w1blk[0:D, c:c + H] = W1[sa]
        w1blk[D:2 * D, c + H:c + 100] = W1[sb]
        w1blk[2 * D, c:c + H] = b1[sa]
        w1blk[2 * D, c + H:c + 100] = b1[sb]
        w2blk[0:H, c:c + H] = W2[sa]
        w2blk[H:2 * H, c + H:c + 100] = W2[sb]
        w2blk[2 * H, c:c + H] = b2[sa]
        w2blk[2 * H, c + H:c + 100] = b2[sb]
    w3img = np.zeros((128, 32 * NPAIR), np.float32)
    for j in range(NPAIR):
        sa, sb = type_list[tidx[j]]
        base = 32 * j
        k = j % POG
        w3img[0:H, base + 2 * k] = W3[sa, :, 0]
        w3img[2 * H, base + 2 * k] = b3[sa, 0]
        w3img[H:2 * H, base + 2 * k + 1] = W3[sb, :, 0]
        w3img[2 * H, base + 2 * k + 1] = b3[sb, 0]

    wmaps = {
        "w1_in": w1blk.astype(NP_MM), "w2_in": w2blk.astype(NP_MM),
        "w3_in": w3img.astype(NP_MM),
    }

    in_maps = []
    for c in range(NCORES):
        d2 = np.ascontiguousarray(
            desc[c * NF:(c + 1) * NF].transpose(1, 2, 0))     # [A, D, NF]
        blk = np.empty((NPAIR, KD, NF), np.float32)
        blk[:, 0:D] = d2[pa]
        blk[:, D:2 * D] = d2[pb]
        blk[:, 2 * D] = 1.0
        dc = (blk.reshape(NDG, PDG, KD, NF).transpose(0, 2, 1, 3)
              .reshape(NDG, KD, PDG * NF)).astype(NP_MM)
        in_maps.append({"desc_in": np.ascontiguousarray(dc), **wmaps})
    return in_maps, pa, pb, tidx, len(type_list)


def kernel(desc, numbers, W1, b1, W2, b2, W3, b3):
    in_maps, pa, pb, tidx, T = _host_inputs(
        desc, numbers, W1, b1, W2, b2, W3, b3)

    nc = _build_program(tidx, T)

    last_err = None
    for _attempt in range(3):
        try:
            res = bass_utils.run_bass_kernel_spmd(
                nc, in_maps, core_ids=list(range(NCORES)))
            break
        except Exception as e:  # transient axon terminal failures
            last_err = e
            import time
            time.sleep(20)
    else:
        raise last_err

    LAST.update(nc=nc, in_maps=in_maps, res=res)

    out = np.empty((N, A), np.float32)
    for c in range(NCORES):
        oc = res.results[c]["out"]                    # [NOG, 32, NF]
        oc = oc.reshape(NOG * POG, 2, NF)             # [pair, (a,b), NF]
        out[c * NF:(c + 1) * NF, pa] = oc[:, 0].T
        out[c * NF:(c + 1) * NF, pb] = oc[:, 1].T
    return out


# revision 22
# speedup vs baseline: 1.5575x; 1.5575x over previous
"""Trainium2 Bass kernel for nn_AtomicNeuralNetwork (species-routed per-atom MLP).

Math (per frame n, atom a with species s = numbers[a]):
    h1 = silu(W1[s].T x + b1[s]);  h2 = silu(W2[s].T h1 + b2[s]);  out = W3[s].T h2 + b3[s]
Shapes: N=4096 frames, A=256 atoms, D_IN=39, H=50, S=8 species.

Strategy:
  - Data parallel over frames: 512 frames per NeuronCore x 8 cores.
  - Host groups atoms into species-pure "packs" of 4 (padding each species
    with duplicate atoms to a multiple of 4; dups discarded on unshard), and
    packs into "groups" of 8 for DMA batching (~640KB per transfer).
  - Same-species consecutive packs are processed as PAIRS sharing [128,2048]
    PSUM tiles so each ScalarE activation covers 8 atoms (halves ACT
    instruction count - ACT is the bottleneck engine).
  - Per pack, the 3 layers run as PE matmuls with the per-species weights
    stationary and frames on the moving axis, packed 4-at-a-time into the
    128x128 array with tile_position (64x64 quadrants for L1/L2; K=50,M=1
    at col positions {0,32,64,96} for L3). Matmuls are emitted in
    row-group-checkerboard order so LDWEIGHTS overlaps the other row
    group's in-flight matmul.
  - silu + bias fused on ScalarE straight out of PSUM.
  - L3 writes into the already-consumed L2 PSUM banks (saves PSUM; the
    shared pool runs 2x [128,2048] slots = all 8 banks).
  - b3 + PSUM evacuation on VectorE into a per-group output tile; one
    strided-partition DMA per group to DRAM.
  - Everything bf16 on the matmul path (PSUM accumulates fp32); desc is
    downcast to bf16 on the host, which also halves the HBM traffic.
"""

import sys

for _p in ("/opt/trn_rl_repo",):
    if _p not in sys.path:
        sys.path.append(_p)

import numpy as np
import ml_dtypes

import concourse.bass as bass  # noqa: F401
import concourse.mybir as mybir
import concourse.tile as tile
from concourse import bacc
from concourse import bass_utils

N, A, D, H, S = 4096, 256, 39, 50, 8
NCORES = 8
NF = N // NCORES            # frames per core
GRP = 8                     # packs per DMA group
MM_DT = mybir.dt.bfloat16
NP_MM = ml_dtypes.bfloat16
PAIRING = False     # pair same-species packs into [128,2048] psum tiles
PS_BUFS = 2 if PAIRING else 4

LAST = {}


def _pack_atoms(species):
    """Group atom indices into species-pure packs. Full packs carry 4 atoms
    (slots 0..3); species leftovers of 1-2 atoms become HALF-packs carrying
    2 real slots (0,1) with filler content in slots 2,3 (width=1 -> the
    device only computes one 512-col half). Returns (slot_atoms [4*NPACK],
    pack_species [NPACK], pack_width [NPACK] in col-halves {1,2},
    slot_valid [4*NPACK])."""
    slot_atoms = []
    pack_species = []
    pack_width = []
    slot_valid = []
    for s in range(S):
        idxs = np.nonzero(species == s)[0].tolist()
        if not idxs:
            continue
        r = len(idxs) % 4
        nfull = len(idxs) // 4
        if r == 3:                       # pad to a full pack
            idxs.append(idxs[-1])
            r = 0
            nfull += 1
        for i in range(0, 4 * nfull, 4):
            slot_atoms.extend(idxs[i:i + 4])
            pack_species.append(s)
            pack_width.append(2)
            slot_valid.extend([True] * 4)
        if r:                            # 1 or 2 leftovers -> half-pack
            a = idxs[4 * nfull]
            b = idxs[4 * nfull + 1] if r == 2 else a
            slot_atoms.extend([a, b, a, b])   # slots 2,3 = filler content
            pack_species.append(s)
            pack_width.append(1)
            slot_valid.extend([True, True, False, False])
    return (np.array(slot_atoms), np.array(pack_species),
            np.array(pack_width), np.array(slot_valid))


def _groups(npack):
    return [(g, min(GRP, npack - g * GRP)) for g in range((npack + GRP - 1) // GRP)]


def _build_program(pack_species, pack_width, npack, repeat=0):
    import contextlib

    nc = bacc.Bacc("TRN2", target_bir_lowering=False, debug=False)

    groups = _groups(npack)
    gn_of = dict(groups)
    ngrp = len(groups)

    desc_in = nc.dram_tensor("desc_in", [ngrp, 2, D, GRP * 2 * NF], MM_DT, kind="ExternalInput")
    w1_in = nc.dram_tensor("w1_in", [128, S * H], MM_DT, kind="ExternalInput")
    w2_in = nc.dram_tensor("w2_in", [128, S * H], MM_DT, kind="ExternalInput")
    w3_in = nc.dram_tensor("w3_in", [128, S], MM_DT, kind="ExternalInput")
    b1_in = nc.dram_tensor("b1_in", [128, S], mybir.dt.float32, kind="ExternalInput")
    b2_in = nc.dram_tensor("b2_in", [128, S], mybir.dt.float32, kind="ExternalInput")
    b3_in = nc.dram_tensor("b3_in", [128, S], mybir.dt.float32, kind="ExternalInput")
    out = nc.dram_tensor("out", [ngrp, 4, GRP, NF], mybir.dt.float32, kind="ExternalOutput")

    Silu = mybir.ActivationFunctionType.Silu
    F32 = mybir.dt.float32

    # pair consecutive same-species packs within each DMA group
    units = []  # (g, [j...]) with 1 or 2 pack-in-group indices
    for g, gn in groups:
        j = 0
        while j < gn:
            p = g * GRP + j
            if PAIRING and j + 1 < gn and pack_species[p + 1] == pack_species[p]:
                units.append((g, [j, j + 1]))
                j += 2
            else:
                units.append((g, [j]))
                j += 1

    with tile.TileContext(nc) as tc:
        with (
            tc.tile_pool(name="const", bufs=1) as cpool,
            tc.tile_pool(name="dt", bufs=6) as dpool,
            tc.tile_pool(name="h1p", bufs=6) as h1pool,
            tc.tile_pool(name="h2p", bufs=6) as h2pool,
            tc.tile_pool(name="op", bufs=3) as opool,
            tc.tile_pool(name="ps", bufs=PS_BUFS, space="PSUM") as pspool,
        ):
            w1 = cpool.tile([128, S * H], MM_DT)
            w2 = cpool.tile([128, S * H], MM_DT)
            w3 = cpool.tile([128, S], MM_DT)
            b1 = cpool.tile([128, S], F32)
            b2 = cpool.tile([128, S], F32)
            b3 = cpool.tile([128, S], F32)
            for t, src in ((w1, w1_in), (w2, w2_in), (w3, w3_in),
                           (b1, b1_in), (b2, b2_in), (b3, b3_in)):
                nc.sync.dma_start(t[:], src[:])

            loop_cm = tc.For_i(0, repeat, 1) if repeat else contextlib.nullcontext()
            with loop_cm:
                def emit_tail(pend):
                    """Deferred L3 + b3/evac (+ group out-DMA) for a finished unit."""
                    (pg, pjs, psp, ph2, pps2, po, pwidth) = pend
                    ssl = slice(psp * H, (psp + 1) * H)
                    if pwidth == 1:
                        j = pjs[0]
                        nc.tensor.matmul(pps2[0:1, 0:NF], w3[0:H, psp:psp + 1],
                                         ph2[0:H, 0:NF],
                                         start=True, stop=True, tile_position=(0, 0))
                        nc.tensor.matmul(pps2[32:33, 0:NF], w3[64:64 + H, psp:psp + 1],
                                         ph2[64:64 + H, 0:NF],
                                         start=True, stop=True, tile_position=(64, 32))
                        nc.vector.tensor_scalar_add(po[:, j * NF:(j + 1) * NF],
                                                    pps2[:, 0:NF], b3[:, psp:psp + 1])
                    else:
                        for k, j in enumerate(pjs):
                            u0 = 2 * k * NF
                            nc.tensor.matmul(pps2[0:1, u0:u0 + NF], w3[0:H, psp:psp + 1],
                                             ph2[0:H, u0:u0 + NF],
                                             start=True, stop=True, tile_position=(0, 0))
                            nc.tensor.matmul(pps2[64:65, u0:u0 + NF], w3[64:64 + H, psp:psp + 1],
                                             ph2[64:64 + H, u0:u0 + NF],
                                             start=True, stop=True, tile_position=(64, 64))
                            nc.tensor.matmul(pps2[32:33, u0:u0 + NF], w3[0:H, psp:psp + 1],
                                             ph2[0:H, u0 + NF:u0 + 2 * NF],
                                             start=True, stop=True, tile_position=(0, 32))
                            nc.tensor.matmul(pps2[96:97, u0:u0 + NF], w3[64:64 + H, psp:psp + 1],
                                             ph2[64:64 + H, u0 + NF:u0 + 2 * NF],
                                             start=True, stop=True, tile_position=(64, 96))
                        j0 = pjs[0]
                        nuu = len(pjs)
                        if nuu == 1:
                            nc.vector.tensor_scalar_add(po[:, j0 * NF:(j0 + 1) * NF],
                                                        pps2[:, 0:NF], b3[:, psp:psp + 1])
                        else:
                            sap = pps2[:, 0:2 * nuu * NF].rearrange(
                                "p (k c) -> p k c", k=nuu)[:, :, 0:NF]
                            dap = po[:, j0 * NF:(j0 + nuu) * NF].rearrange(
                                "p (k c) -> p k c", k=nuu)
                            nc.vector.tensor_scalar_add(dap, sap, b3[:, psp:psp + 1])
                    if pjs[-1] == gn_of[pg] - 1:
                        gnn = gn_of[pg]
                        sap = po[:, 0:gnn * NF].rearrange(
                            "(a p) (j f) -> a p j f", p=32, f=NF)[:, 0]
                        nc.sync.dma_start(out[pg, :, 0:gnn, :], sap)

                cur_g = -1
                dt_t = None
                o = None
                pending = None
                for (g, js) in units:
                    if g != cur_g:
                        cur_g = g
                        gn = gn_of[g]
                        gw = gn * 2 * NF
                        dt_t = dpool.tile([128, GRP * 2 * NF], MM_DT, tag="dt")
                        nc.sync.dma_start(dt_t[0:D, 0:gw], desc_in[g, 0, :, 0:gw])
                        nc.gpsimd.dma_start(dt_t[64:64 + D, 0:gw], desc_in[g, 1, :, 0:gw])
                        o = opool.tile([128, GRP * NF], F32, tag="o")

                    p0 = g * GRP + js[0]
                    s = int(pack_species[p0])
                    sl = slice(s * H, (s + 1) * H)
                    nu = len(js)
                    if nu == 1 and pack_width[p0] == 1:
                        # ---- half-pack: 2 atoms, one 512-col psum bank/layer ----
                        j = js[0]
                        c0 = 2 * j * NF
                        ps1 = pspool.tile([128, 2 * NF], F32, tag="ps")
                        nc.tensor.matmul(ps1[0:H, 0:NF], w1[0:D, sl],
                                         dt_t[0:D, c0:c0 + NF],
                                         start=True, stop=True, tile_position=(0, 0))
                        nc.tensor.matmul(ps1[64:64 + H, 0:NF], w1[0:D, sl],
                                         dt_t[0:D, c0 + NF:c0 + 2 * NF],
                                         start=True, stop=True, tile_position=(0, 64))
                        h1 = h1pool.tile([128, 2 * NF], MM_DT, tag="h1")
                        nc.scalar.activation(h1[:, 0:NF], ps1[:, 0:NF], Silu,
                                             bias=b1[:, s:s + 1])
                        ps2 = pspool.tile([128, 2 * NF], F32, tag="ps")
                        nc.tensor.matmul(ps2[0:H, 0:NF], w2[0:H, sl], h1[0:H, 0:NF],
                                         start=True, stop=True, tile_position=(0, 0))
                        nc.tensor.matmul(ps2[64:64 + H, 0:NF], w2[64:64 + H, sl],
                                         h1[64:64 + H, 0:NF],
                                         start=True, stop=True, tile_position=(64, 64))
                        h2 = h2pool.tile([128, 2 * NF], MM_DT, tag="h2")
                        nc.scalar.activation(h2[:, 0:NF], ps2[:, 0:NF], Silu,
                                             bias=b2[:, s:s + 1])
                        if pending is not None:
                            emit_tail(pending)
                        pending = (g, js, s, h2, ps2, o, 1)
                        continue
                    uw = nu * 2 * NF                       # unit width in psum cols

                    # ---- L1: 4 MMs per pack, checkerboard row order ----
                    ps1 = pspool.tile([128, 2 * nu * NF], F32, tag="ps")
                    for k, j in enumerate(js):
                        c0 = 2 * j * NF
                        u0 = 2 * k * NF
                        nc.tensor.matmul(ps1[0:H, u0:u0 + NF], w1[0:D, sl],
                                         dt_t[0:D, c0:c0 + NF],
                                         start=True, stop=True, tile_position=(0, 0))
                        nc.tensor.matmul(ps1[0:H, u0 + NF:u0 + 2 * NF], w1[64:64 + D, sl],
                                         dt_t[64:64 + D, c0:c0 + NF],
                                         start=True, stop=True, tile_position=(64, 0))
                        nc.tensor.matmul(ps1[64:64 + H, u0:u0 + NF], w1[0:D, sl],
                                         dt_t[0:D, c0 + NF:c0 + 2 * NF],
                                         start=True, stop=True, tile_position=(0, 64))
                        nc.tensor.matmul(ps1[64:64 + H, u0 + NF:u0 + 2 * NF], w1[64:64 + D, sl],
                                         dt_t[64:64 + D, c0 + NF:c0 + 2 * NF],
                                         start=True, stop=True, tile_position=(64, 64))

                    h1 = h1pool.tile([128, 2 * nu * NF], MM_DT, tag="h1")
                    nc.scalar.activation(h1[:, 0:uw], ps1[:, 0:uw], Silu, bias=b1[:, s:s + 1])

                    # ---- L2 ----
                    ps2 = pspool.tile([128, 2 * nu * NF], F32, tag="ps")
                    for k, j in enumerate(js):
                        u0 = 2 * k * NF
                        nc.tensor.matmul(ps2[0:H, u0:u0 + NF], w2[0:H, sl],
                                         h1[0:H, u0:u0 + NF],
                                         start=True, stop=True, tile_position=(0, 0))
                        nc.tensor.matmul(ps2[0:H, u0 + NF:u0 + 2 * NF], w2[64:64 + H, sl],
                                         h1[64:64 + H, u0:u0 + NF],
                                         start=True, stop=True, tile_position=(64, 0))
                        nc.tensor.matmul(ps2[64:64 + H, u0:u0 + NF], w2[0:H, sl],
                                         h1[0:H, u0 + NF:u0 + 2 * NF],
                                         start=True, stop=True, tile_position=(0, 64))
                        nc.tensor.matmul(ps2[64:64 + H, u0 + NF:u0 + 2 * NF], w2[64:64 + H, sl],
                                         h1[64:64 + H, u0 + NF:u0 + 2 * NF],
                                         start=True, stop=True, tile_position=(64, 64))

                    h2 = h2pool.tile([128, 2 * nu * NF], MM_DT, tag="h2")
                    nc.scalar.activation(h2[:, 0:uw], ps2[:, 0:uw], Silu, bias=b2[:, s:s + 1])

                    if pending is not None:
                        emit_tail(pending)
                    pending = (g, js, s, h2, ps2, o, 2)

                if pending is not None:
                    emit_tail(pending)

    nc.compile()
    return nc


def _host_inputs(desc, numbers, W1, b1, W2, b2, W3, b3):
    desc = np.asarray(desc, dtype=np.float32)
    numbers = np.asarray(numbers).astype(np.int64)
    W1 = np.asarray(W1, np.float32); b1 = np.asarray(b1, np.float32)
    W2 = np.asarray(W2, np.float32); b2 = np.asarray(b2, np.float32)
    W3 = np.asarray(W3, np.float32); b3 = np.asarray(b3, np.float32)

    slot_atoms, pack_species, pack_width, slot_valid = _pack_atoms(numbers)
    npack = len(pack_species)
    nslot = 4 * npack
    groups = _groups(npack)
    ngrp = len(groups)

    w1img = np.zeros((128, S * H), np.float32)
    w2img = np.zeros((128, S * H), np.float32)
    w3img = np.zeros((128, S), np.float32)
    b1img = np.zeros((128, S), np.float32)
    b2img = np.zeros((128, S), np.float32)
    b3img = np.zeros((128, S), np.float32)
    for s in range(S):
        sl = slice(s * H, (s + 1) * H)
        w1img[0:D, sl] = W1[s]; w1img[64:64 + D, sl] = W1[s]
        w2img[0:H, sl] = W2[s]; w2img[64:64 + H, sl] = W2[s]
        w3img[0:H, s] = W3[s, :, 0]; w3img[64:64 + H, s] = W3[s, :, 0]
        b1img[0:H, s] = b1[s]; b1img[64:64 + H, s] = b1[s]
        b2img[0:H, s] = b2[s]; b2img[64:64 + H, s] = b2[s]
        b3img[[0, 32, 64, 96], s] = b3[s, 0]

    wmaps = {
        "w1_in": w1img.astype(NP_MM), "w2_in": w2img.astype(NP_MM),
        "w3_in": w3img.astype(NP_MM),
        "b1_in": b1img, "b2_in": b2img, "b3_in": b3img,
    }

    npack_pad = ngrp * GRP
    in_maps = []
    for c in range(NCORES):
        dc = desc[c * NF:(c + 1) * NF][:, slot_atoms, :]                  # [NF, NSLOT, D]
        dc = np.ascontiguousarray(dc.transpose(1, 2, 0)).astype(NP_MM)    # [NSLOT, D, NF]
        if npack_pad != npack:
            pad = np.zeros((4 * (npack_pad - npack), D, NF), NP_MM)
            dc = np.concatenate([dc, pad], axis=0)
        dc = dc.reshape(ngrp, GRP, 2, 2, D, NF)      # g, j, rh, ch, q, n
        dc = dc.transpose(0, 2, 4, 1, 3, 5)          # g, rh, q, j, ch, n
        dc = np.ascontiguousarray(dc).reshape(ngrp, 2, D, GRP * 2 * NF)
        in_maps.append({"desc_in": dc, **wmaps})
    return in_maps, slot_atoms, pack_species, pack_width, slot_valid, npack, nslot


def kernel(desc, numbers, W1, b1, W2, b2, W3, b3):
    (in_maps, slot_atoms, pack_species, pack_width, slot_valid,
     npack, nslot) = _host_inputs(desc, numbers, W1, b1, W2, b2, W3, b3)

    nc = _build_program(pack_species, pack_width, npack)

    last_err = None
    for _attempt in range(3):
        try:
            res = bass_utils.run_bass_kernel_spmd(
                nc, in_maps, core_ids=list(range(NCORES)))
            break
        except Exception as e:  # transient axon terminal failures
            last_err = e
            import time
            time.sleep(20)
    else:
        raise last_err

    LAST.update(nc=nc, in_maps=in_maps, res=res, npack=npack)

    out = np.empty((N, A), np.float32)
    for c in range(NCORES):
        oc = res.results[c]["out"]                     # [ngrp, 4, GRP, NF]
        oc = oc.transpose(0, 2, 1, 3).reshape(-1, NF)  # slot-major
        out[c * NF:(c + 1) * NF, slot_atoms[slot_valid]] = oc[:nslot][slot_valid].T
    return out

